# revision 2
# baseline (speedup 1.0000x reference)
import sys

sys.path.insert(0, "/opt/trn_rl_repo")

import numpy as np

import concourse.bacc as bacc
import concourse.bass as bass
import concourse.mybir as mybir
from concourse import tile
from concourse.bass_utils import run_bass_kernel_spmd

N, M, D = 1024, 1024, 256
N_CORES = 8
NP = N // N_CORES
P = 128
KC = D // P
DT = mybir.dt.float32
BF = mybir.dt.bfloat16
F32 = np.float32

OMEGA = [0.39780104230249563, 1.210745875305645, 2.1045022695431936]
BCOEF = [1.1997669693853161, 0.25283979853695815, 0.07651384344785625]
J = 3
TWO_PI = float(2.0 * np.pi)
MAGIC = float(1.5 * 2.0**23)

_CACHE = {}


def build_nc():
    nc = bacc.Bacc("TRN2", target_bir_lowering=False, debug=False, num_devices=N_CORES)

    frT = nc.declare_dram_parameter("frT", [D, NP], BF, isOutput=False)
    fpT = nc.declare_dram_parameter("fpT", [D, M], BF, isOutput=False)
    fp = nc.declare_dram_parameter("fp", [M, D], BF, isOutput=False)
    WwT = nc.declare_dram_parameter("WwT", [D, D], BF, isOutput=False)
    WpT = nc.declare_dram_parameter("WpT", [D, D], BF, isOutput=False)
    Wb = nc.declare_dram_parameter("Wb", [D, 1], DT, isOutput=False)
    Wpb = nc.declare_dram_parameter("Wpb", [D, 1], DT, isOutput=False)
    WBbig = nc.declare_dram_parameter("WBbig", [P, 2 * J * KC * NP], DT, isOutput=False)
    wpB = nc.declare_dram_parameter("wpB", [P, D], DT, isOutput=False)
    ident = nc.declare_dram_parameter("ident", [P, P], DT, isOutput=False)

    ctx_out = nc.declare_dram_parameter("ctx_out", [NP, D], DT, isOutput=True)
    s_out = nc.declare_dram_parameter("s_out", [NP, 1], DT, isOutput=True)

    QW = KC * NP

    with tile.TileContext(nc) as tc:
        with (
            tc.tile_pool(name="const", bufs=1) as cpool,
            tc.tile_pool(name="feat", bufs=1) as fpool,
            tc.tile_pool(name="work", bufs=2) as wpool,
            tc.tile_pool(name="small", bufs=2) as spool,
            tc.tile_pool(name="alpha", bufs=4) as apool,
            tc.tile_pool(name="ps_qk", bufs=2, space="PSUM") as ps_qk,
            tc.tile_pool(name="ps_s", bufs=1, space="PSUM") as ps_s,
            tc.tile_pool(name="ps_tr", bufs=2, space="PSUM") as ps_tr,
            tc.tile_pool(name="ps_ctx", bufs=1, space="PSUM") as ps_ctx,
        ):
            pihalf = cpool.tile([P, 1], DT, name="pihalf")
            nc.vector.memset(pihalf[:, :], float(np.pi / 2))
            scratch = cpool.tile([1, 2], DT, name="scratch")
            nc.vector.memset(scratch[:, :], 0.0)
            nc.scalar.activation(
                scratch[:, :], scratch[:, :], mybir.ActivationFunctionType.Sin
            )

            WpT_sb = [cpool.tile([P, D], BF, name=f"WpT{k}") for k in range(KC)]
            fpT_sb = [cpool.tile([P, M], BF, name=f"fpT{k}") for k in range(KC)]
            Wpb_sb = [cpool.tile([P, 1], DT, name=f"Wpb{k}") for k in range(KC)]
            frT_sb = [cpool.tile([P, NP], BF, name=f"frT{k}") for k in range(KC)]
            WwT_sb = [cpool.tile([P, D], BF, name=f"WwT{k}") for k in range(KC)]
            Wb_sb = [cpool.tile([P, 1], DT, name=f"Wb{k}") for k in range(KC)]
            for k in range(KC):
                nc.sync.dma_start(out=WpT_sb[k][:, :], in_=WpT[k * P : (k + 1) * P, :])
                nc.gpsimd.dma_start(
                    out=fpT_sb[k][:, :], in_=fpT[k * P : (k + 1) * P, :]
                )
            nc.sync.dma_start(out=Wpb_sb[0][:, :], in_=Wpb[0:P, :])
            nc.gpsimd.dma_start(out=Wpb_sb[1][:, :], in_=Wpb[P : 2 * P, :])
            for k in range(KC):
                nc.sync.dma_start(out=frT_sb[k][:, :], in_=frT[k * P : (k + 1) * P, :])
                nc.gpsimd.dma_start(
                    out=WwT_sb[k][:, :], in_=WwT[k * P : (k + 1) * P, :]
                )
            nc.sync.dma_start(out=Wb_sb[0][:, :], in_=Wb[0:P, :])
            nc.gpsimd.dma_start(out=Wb_sb[1][:, :], in_=Wb[P : 2 * P, :])
            WBbig_sb = cpool.tile([P, 2 * J * QW], DT, name="WBbig")
            nc.sync.dma_start(out=WBbig_sb[:, :], in_=WBbig[:, :])
            fp_sb = []
            for mj in range(M // P):
                t_fp = cpool.tile([P, D], BF, name=f"fp{mj}")
                (nc.sync if mj % 2 else nc.gpsimd).dma_start(
                    out=t_fp[:, :], in_=fp[mj * P : (mj + 1) * P, :]
                )
                fp_sb.append(t_fp)
            wpB_sb = cpool.tile([P, D], DT, name="wpB")
            nc.sync.dma_start(out=wpB_sb[:, :], in_=wpB[:, :])
            ident_sb = cpool.tile([P, P], DT, name="ident")
            nc.gpsimd.dma_start(out=ident_sb[:, :], in_=ident[:, :])

            kT_sb = cpool.tile([P, KC * M], DT, name="kT")
            qT_sb = cpool.tile([P, QW], DT, name="qT")
            for c in range(KC):
                for h in range(M // 512):
                    k_ps = ps_qk.tile([P, 512], DT, name="k_ps", tag="qk")
                    for k in range(KC):
                        nc.tensor.matmul(
                            k_ps[:, :],
                            lhsT=WpT_sb[k][:, c * P : (c + 1) * P],
                            rhs=fpT_sb[k][:, h * 512 : (h + 1) * 512],
                            start=(k == 0),
                            stop=(k == KC - 1),
                        )
                    nc.vector.tensor_scalar_add(
                        kT_sb[:, c * M + h * 512 : c * M + (h + 1) * 512],
                        k_ps[:, :],
                        Wpb_sb[c][:, 0:1],
                    )
            for c in range(KC):
                q_ps = ps_qk.tile([P, 512], DT, name="q_ps", tag="qk")
                for k in range(KC):
                    nc.tensor.matmul(
                        q_ps[:, :NP],
                        lhsT=WwT_sb[k][:, c * P : (c + 1) * P],
                        rhs=frT_sb[k][:, :],
                        start=(k == 0),
                        stop=(k == KC - 1),
                    )
                nc.vector.tensor_scalar_add(
                    qT_sb[:, c * NP : (c + 1) * NP], q_ps[:, :NP], Wb_sb[c][:, 0:1]
                )

            Ks = [fpool.tile([P, KC * M], BF, name=f"Ks{j}") for j in range(J)]
            Kc = [fpool.tile([P, KC * M], BF, name=f"Kc{j}") for j in range(J)]
            Qraw = wpool.tile([P, 2 * J * QW], DT, name="Qraw", bufs=1)

            k_us, k_uc = {}, {}
            for j in range(1, J):
                cj = float(OMEGA[j] / TWO_PI)
                t_k = fpool.tile([P, KC * M], DT, name=f"t_k{j}")
                nc.vector.tensor_scalar_mul(t_k[:, :], kT_sb[:, :], cj)
                r_s = fpool.tile([P, KC * M], DT, name=f"r_s{j}")
                nc.vector.tensor_scalar(
                    r_s[:, :], t_k[:, :], MAGIC, MAGIC,
                    mybir.AluOpType.add, mybir.AluOpType.subtract,
                )
                u_s = fpool.tile([P, KC * M], DT, name=f"u_s{j}")
                nc.vector.tensor_tensor(
                    u_s[:, :], t_k[:, :], r_s[:, :], mybir.AluOpType.subtract
                )
                v_k = fpool.tile([P, KC * M], DT, name=f"v_k{j}")
                nc.vector.tensor_scalar(
                    v_k[:, :], t_k[:, :], 0.25, MAGIC,
                    mybir.AluOpType.add, mybir.AluOpType.add,
                )
                r_c = fpool.tile([P, KC * M], DT, name=f"r_c{j}")
                nc.vector.tensor_scalar(
                    r_c[:, :], v_k[:, :], MAGIC, None,
                    mybir.AluOpType.subtract,
                )
                u_c = fpool.tile([P, KC * M], DT, name=f"u_c{j}")
                nc.vector.tensor_tensor(
                    u_c[:, :], t_k[:, :], r_c[:, :], mybir.AluOpType.subtract
                )
                k_us[j], k_uc[j] = u_s, u_c
            q_us, q_uc = {}, {}
            for j in range(1, J):
                cj = float(OMEGA[j] / TWO_PI)
                t_q = fpool.tile([P, QW], DT, name=f"t_q{j}")
                nc.vector.tensor_scalar_mul(t_q[:, :], qT_sb[:, :], cj)
                rq_s = fpool.tile([P, QW], DT, name=f"rq_s{j}")
                nc.vector.tensor_scalar(
                    rq_s[:, :], t_q[:, :], MAGIC, MAGIC,
                    mybir.AluOpType.add, mybir.AluOpType.subtract,
                )
                uq_s = fpool.tile([P, QW], DT, name=f"uq_s{j}")
                nc.vector.tensor_tensor(
                    uq_s[:, :], t_q[:, :], rq_s[:, :], mybir.AluOpType.subtract
                )
                vq = fpool.tile([P, QW], DT, name=f"vq{j}")
                nc.vector.tensor_scalar(
                    vq[:, :], t_q[:, :], 0.25, MAGIC,
                    mybir.AluOpType.add, mybir.AluOpType.add,
                )
                rq_c = fpool.tile([P, QW], DT, name=f"rq_c{j}")
                nc.vector.tensor_scalar(
                    rq_c[:, :], vq[:, :], MAGIC, None,
                    mybir.AluOpType.subtract,
                )
                uq_c = fpool.tile([P, QW], DT, name=f"uq_c{j}")
                nc.vector.tensor_tensor(
                    uq_c[:, :], t_q[:, :], rq_c[:, :], mybir.AluOpType.subtract
                )
                q_us[j], q_uc[j] = uq_s, uq_c
            Sin = mybir.ActivationFunctionType.Sin
            phi = fpool.tile([P, 2 * J * QW], BF, name="phi")
            for j in range(J):
                om = float(OMEGA[j])
                sb = slice((2 * j) * QW, (2 * j + 1) * QW)
                cb = slice((2 * j + 1) * QW, (2 * j + 2) * QW)
                jb = slice((2 * j) * QW, (2 * j + 2) * QW)
                if j == 0:
                    nc.scalar.activation(Qraw[:, sb], qT_sb[:, :], Sin, scale=om)
                    nc.scalar.activation(
                        Qraw[:, cb], qT_sb[:, :], Sin,
                        bias=pihalf[:, 0:1], scale=om,
                    )
                else:
                    nc.scalar.activation(Qraw[:, sb], q_us[j][:, :], Sin, scale=TWO_PI)
                    nc.scalar.activation(
                        Qraw[:, cb], q_uc[j][:, :], Sin,
                        bias=pihalf[:, 0:1], scale=TWO_PI,
                    )
                nc.vector.tensor_tensor(
                    phi[:, jb], Qraw[:, jb], WBbig_sb[:, jb], mybir.AluOpType.mult
                )
                if j == 0:
                    nc.scalar.activation(Ks[j][:, :], kT_sb[:, :], Sin, scale=om)
                    nc.scalar.activation(
                        Kc[j][:, :], kT_sb[:, :], Sin,
                        bias=pihalf[:, 0:1], scale=om,
                    )
                else:
                    nc.scalar.activation(Ks[j][:, :], k_us[j][:, :], Sin, scale=TWO_PI)
                    nc.scalar.activation(
                        Kc[j][:, :], k_uc[j][:, :], Sin,
                        bias=pihalf[:, 0:1], scale=TWO_PI,
                    )

            S_ps = ps_s.tile([P, M], DT, name="S_ps")
            mm = []
            for j in range(J):
                mm.append(((2 * j) * QW, Kc[j]))
                mm.append(((2 * j + 1) * QW, Ks[j]))
            n_mm = len(mm) * KC
            idx = 0
            for col0, Kmap in mm:
                for c in range(KC):
                    for h in range(M // 512):
                        nc.tensor.matmul(
                            S_ps[:, h * 512 : (h + 1) * 512],
                            lhsT=phi[:, col0 + c * NP : col0 + (c + 1) * NP],
                            rhs=Kmap[:, c * M + h * 512 : c * M + (h + 1) * 512],
                            start=(idx == 0),
                            stop=(idx == n_mm - 1),
                        )
                    idx += 1

            expS = wpool.tile([P, M], DT, name="expS", bufs=1)
            sumex = spool.tile([P, 2], DT, name="sumex")
            for h in range(2):
                nc.scalar.activation(
                    expS[:, h * 512 : (h + 1) * 512],
                    S_ps[:, h * 512 : (h + 1) * 512],
                    mybir.ActivationFunctionType.Exp,
                    accum_out=sumex[:, h : h + 1],
                )
            sumt = spool.tile([P, 1], DT, name="sumt")
            nc.vector.tensor_add(sumt[:, :], sumex[:, 0:1], sumex[:, 1:2])
            rs = spool.tile([P, 1], DT, name="rs")
            nc.vector.reciprocal(rs[:, :], sumt[:, :])

            ctx_ps = ps_ctx.tile([P, D], DT, name="ctx_ps")
            for mj in range(M // P):
                tr_ps = ps_tr.tile([P, P], DT, name="tr_ps")
                nc.tensor.transpose(
                    tr_ps[:, :], expS[:, mj * P : (mj + 1) * P], ident_sb[:, :]
                )
                aT = apool.tile([P, P], BF, name="aT")
                nc.vector.tensor_copy(aT[:, :], tr_ps[:, :])
                nc.tensor.matmul(
                    ctx_ps[:, :],
                    lhsT=aT[:, :],
                    rhs=fp_sb[mj][:, :],
                    start=(mj == 0),
                    stop=(mj == M // P - 1),
                )
            ctx_sb = wpool.tile([P, D], DT, name="ctx_sb", bufs=1)
            nc.vector.tensor_scalar_mul(ctx_sb[:, :], ctx_ps[:, :], rs[:, 0:1])

            tmp = wpool.tile([P, D], DT, name="tmp", bufs=1)
            nc.vector.tensor_mul(tmp[:, :], ctx_sb[:, :], wpB_sb[:, :])
            s_sb = spool.tile([P, 1], DT, name="s_sb")
            nc.vector.reduce_sum(s_sb[:, :], tmp[:, :], axis=mybir.AxisListType.X)

            nc.sync.dma_start(out=ctx_out[:, :], in_=ctx_sb[:, :])
            nc.sync.dma_start(out=s_out[:, :], in_=s_sb[:, :])

    nc.finalize()
    return nc


def _prep_inputs(f_r, f_r_prime, W_w, W_b, Wp_w, Wp_b, w_w, w_b, wp_w, wp_b):
    import ml_dtypes

    BF_NP = ml_dtypes.bfloat16
    fpT = np.ascontiguousarray(f_r_prime.T).astype(BF_NP)
    fp = np.ascontiguousarray(f_r_prime).astype(BF_NP)
    WwT = np.ascontiguousarray(W_w.T).astype(BF_NP)
    WpT = np.ascontiguousarray(Wp_w.T).astype(BF_NP)
    Wb = np.ascontiguousarray(W_b.reshape(D, 1), dtype=F32)
    Wpb = np.ascontiguousarray(Wp_b.reshape(D, 1), dtype=F32)
    w = w_w.reshape(KC, P)
    WBbig = np.empty((P, 2 * J * KC * NP), dtype=F32)
    for j in range(J):
        for f in range(2):
            for c in range(KC):
                col0 = (2 * j + f) * KC * NP + c * NP
                WBbig[:, col0 : col0 + NP] = (w[c] * BCOEF[j])[:, None]
    wpB = np.broadcast_to(wp_w.reshape(1, D), (P, D)).astype(F32).copy()
    ident = np.eye(P, dtype=F32)

    shared = {
        "fpT": fpT,
        "fp": fp,
        "WwT": WwT,
        "WpT": WpT,
        "Wb": Wb,
        "Wpb": Wpb,
        "WBbig": WBbig,
        "wpB": wpB,
        "ident": ident,
    }
    in_maps = []
    for c in range(N_CORES):
        frT = np.ascontiguousarray(f_r[c * NP : (c + 1) * NP, :].T).astype(BF_NP)
        in_maps.append({"frT": frT, **shared})
    return in_maps


def _run(in_maps, **kw):
    if "nc" not in _CACHE:
        _CACHE["nc"] = build_nc()
    return run_bass_kernel_spmd(_CACHE["nc"], in_maps, list(range(N_CORES)), **kw)


def kernel(f_r, f_r_prime, W_w, W_b, Wp_w, Wp_b, w_w, w_b, wp_w, wp_b):
    in_maps = _prep_inputs(
        f_r, f_r_prime, W_w, W_b, Wp_w, Wp_b, w_w, w_b, wp_w, wp_b
    )
    res = _run(in_maps)
    ctx = np.concatenate([res.results[c]["ctx_out"] for c in range(N_CORES)], axis=0)
    s = np.concatenate(
        [res.results[c]["s_out"][:, 0] for c in range(N_CORES)], axis=0
    ).astype(np.float64)
    s -= s.max()
    e = np.exp(s)
    a = (e / e.sum()).astype(F32)
    pool = a[None, :] @ ctx
    return pool.astype(F32)
